# revision 27
# baseline (speedup 1.0000x reference)
"""GQA attention block (Wq/Wk/Wv -> RoPE -> softmax(QK^T)V -> Wo) on 8 Trainium2
NeuronCores.

Sharding (tensor-parallel per the head-sharding scheme):
  core c in 0..7: batch b = c // 4, head-group g = c % 4.
  Each core owns 8 q-heads (global 8g..8g+7) and 2 kv-heads (2g, 2g+1) of one
  batch element, computes its slice of q/k/v projections, RoPE, attention, and
  a partial o_proj (Wo rows for its heads). The all-reduce after o_proj is the
  host-side unshard: out[b] = sum of the 4 partial outputs of batch b.

On-device layout (per core), everything feature-on-partitions ("transposed"):
  xt    [D=2048, S=2048]   x^T for this batch
  QT    [E=512,  S]        q^T; partition-tile j holds head pair (j, j+4):
                           local head j (kv0) on partitions 0:64, head j+4
                           (kv1) on partitions 64:128. Wq columns are permuted
                           on the host to produce this layout directly.
  KT    [128, S]           k^T; kv0 on partitions 0:64, kv1 on 64:128.
  V     [S, 130] as 16 tiles [128, 130]: cols 0:64 v(kv0), col 64 ones,
                           cols 65:129 v(kv1), col 129 ones  (v_aug).
  scores^T per head: [sk, sq] so exp is ACT psum->sbuf and the attn@v
  contraction (over sk) uses v_aug as the stationary operand; row 64 of the
  attn@v output is the softmax denominator (ones column trick).

Matmuls run as float32r (full-rate fp32 on the PE; ~1.6e-4 rel err measured).
"""

import sys

if "/opt/trn_rl_repo" not in sys.path:
    sys.path.insert(0, "/opt/trn_rl_repo")

from contextlib import ExitStack

import numpy as np
import ml_dtypes

import concourse.bass as bass  # noqa: F401  (engine types via nc)
import concourse.tile as tile
from concourse import bacc, bass_utils, mybir

F32 = mybir.dt.float32
F32R = mybir.dt.float32r
BF16 = mybir.dt.bfloat16
AF = mybir.ActivationFunctionType

# Problem constants (hardcoded per harness contract)
B = 2
S = 2048  # sequence length
D = 2048  # d_model
N_HEADS = 32
N_KV = 8
HD = 64  # head dim
ROPE_BASE = 500000.0
N_CORES = 8

# Per-core derived
NQ = N_HEADS // 4  # 8 local q heads (4 head-groups)
E = NQ * HD  # 512 local q features
NPAIR = NQ // 2  # 4 head pairs / e-tiles
KVW = 2 * HD  # 128 local kv features
SC = 512  # s-chunk (projection + sq chunk)
NSC = S // SC  # 4
DT = D // 128  # 16 d-tiles
SKT = S // 128  # 16 sk tiles
ET = E // 128  # 4 e-tiles
SCALE = 1.0 / float(np.sqrt(HD))


def build_program():
    nc = bacc.Bacc(
        "TRN2", target_bir_lowering=False, debug=False, enable_asserts=False
    )

    xt = nc.dram_tensor("xt", [D, S], BF16, kind="ExternalInput").ap()
    wq = nc.dram_tensor("wq", [D, E], BF16, kind="ExternalInput").ap()
    wk = nc.dram_tensor("wk", [D, KVW], BF16, kind="ExternalInput").ap()
    wv = nc.dram_tensor("wv", [D, KVW], BF16, kind="ExternalInput").ap()
    wo = nc.dram_tensor("wo", [E, D], BF16, kind="ExternalInput").ap()
    cosd = nc.dram_tensor("cosd", [128, S], F32R, kind="ExternalInput").ap()
    sind = nc.dram_tensor("sind", [128, S], F32R, kind="ExternalInput").ap()
    ident = nc.dram_tensor("ident", [128, 128], F32, kind="ExternalInput").ap()
    ones1 = nc.dram_tensor("ones1", [1, 128], F32R, kind="ExternalInput").ap()
    onesc = nc.dram_tensor("onesc", [128, 1], F32R, kind="ExternalInput").ap()
    out = nc.dram_tensor("out", [S, D], F32, kind="ExternalOutput").ap()

    with tile.TileContext(nc) as tc, ExitStack() as ctx:
        persist = ctx.enter_context(tc.tile_pool(name="persist", bufs=1))

        # Persistent SBUF state
        qt_sb = [persist.tile([128, S], F32R, tag=f"qt{j}", name=f"qt{j}") for j in range(NPAIR)]
        kt_sb = persist.tile([128, S], F32R, tag="kt")
        v_sb = [persist.tile([128, 130], F32R, tag=f"v{j}", name=f"v{j}") for j in range(SKT)]
        onesc_sb = persist.tile([128, 1], F32R, tag="onesc")
        nc.sync.dma_start(out=onesc_sb, in_=onesc)
        ones1_sb = persist.tile([1, 128], F32R, tag="ones1")
        nc.sync.dma_start(out=ones1_sb, in_=ones1)

        # ---------------- Phase 1: projections + RoPE + V transpose -------------
        with (
            tc.tile_pool(name="xtp", bufs=3) as xtp,
            tc.tile_pool(name="wp", bufs=1) as wp,
            tc.tile_pool(name="ropec", bufs=1) as ropec,
            tc.tile_pool(name="p1st", bufs=3) as p1st,
            tc.tile_pool(name="qt_ps", bufs=2, space="PSUM") as qt_ps,
            tc.tile_pool(name="kv_ps", bufs=1, space="PSUM") as kv_ps,
            tc.tile_pool(name="tr_ps", bufs=2, space="PSUM") as tr_ps,
        ):

            wq_r = wq.rearrange("(t p) e -> p t e", p=128)
            wk_r = wk.rearrange("(t p) e -> p t e", p=128)
            wv_r = wv.rearrange("(t p) e -> p t e", p=128)
            wq_sb = wp.tile([128, DT, E], BF16, tag="wq")
            wk_sb = wp.tile([128, DT, KVW], BF16, tag="wk")
            wv_sb = wp.tile([128, DT, KVW], BF16, tag="wv")
            xt_r = xt.rearrange("(t p) s -> p t s", p=128)
            xt_c0 = xtp.tile([128, DT, SC], BF16, tag="xt", name="xt_c0")
            # wq via gpsimd SWDGE immediately (own queue): Q-proj of chunk 0
            # starts ~12us in and must not queue behind cos/sin on the ACT ring.
            nc.gpsimd.dma_start(out=wq_sb, in_=wq_r)
            nc.gpsimd.dma_start(out=wv_sb, in_=wv_r)
            ident_sb = ropec.tile([128, 128], F32, tag="ident")
            nc.scalar.dma_start(out=ident_sb, in_=ident)
            cos_sb = ropec.tile([128, S], F32R, tag="cos")
            nc.scalar.dma_start(out=cos_sb, in_=cosd)
            sin_sb = ropec.tile([128, S], F32R, tag="sin")
            nc.scalar.dma_start(out=sin_sb, in_=sind)
            # xt+wk interleaved per-tile: K-projection of chunk 0 unblocks
            # fastest.
            for t in range(DT):
                nc.sync.dma_start(out=xt_c0[:, t, :], in_=xt_r[:, t, 0:SC])
                nc.sync.dma_start(out=wk_sb[:, t, :], in_=wk_r[:, t, :])


            SHUF_MASK = [(i + 16) % 32 for i in range(32)]

            def rope(dst, src_ps, cs, raw_tag):
                """dst[:, cs*SC:+SC] = src_ps*cos + shuffle(src)*sin_signed.

                Features are laid out (host-side permutation) so the RoPE
                rotate pairing is a +-16 swap within each 32-partition
                quadrant; the rotate sign is folded into sind."""
                sl = bass.ts(cs, SC)
                raw = p1st.tile([128, SC], F32R, tag="raw", name="raw", bufs=3)
                nc.scalar.copy(raw, src_ps)
                rp = p1st.tile([128, SC], F32, tag="shuf", name="shuf", bufs=3)
                nc.vector.stream_shuffle(rp, raw, SHUF_MASK)
                tcos = p1st.tile([128, SC], F32, tag="tmp", name="tcos", bufs=4)
                nc.vector.tensor_mul(tcos, raw, cos_sb[:, sl])
                tsin = p1st.tile([128, SC], F32, tag="tmp", name="tsin", bufs=4)
                nc.vector.tensor_mul(tsin, rp, sin_sb[:, sl])
                nc.vector.tensor_add(dst[:, sl], tcos, tsin)

            def qproj(cs, xt_tile):
                # QT projection + rope, per e-tile (head pair)
                for j in range(NPAIR):
                    qp = qt_ps.tile([128, SC], F32, tag="qt")
                    for t in range(DT):
                        nc.tensor.matmul(
                            qp,
                            wq_sb[:, t, bass.ts(j, 128)],
                            xt_tile[:, t, :],
                            start=(t == 0),
                            stop=(t == DT - 1),
                        )
                    rope(qt_sb[j], qp, cs, "qraw")

            # Q projections run one chunk behind K/V: Q(cs-1) fills the PE
            # while chunk cs's xt DMA streams in (and chunk 0's Q no longer
            # races the wq DMA).
            xt_tiles = [xt_c0]
            for cs in range(NSC):
                if cs > 0:
                    # single batched DMA per chunk: latency hidden behind
                    # the interleaved Q(cs-1) work
                    xt_t = xtp.tile([128, DT, SC], BF16, tag="xt")
                    nc.sync.dma_start(
                        out=xt_t, in_=xt_r[:, :, bass.ts(cs, SC)]
                    )
                    xt_tiles.append(xt_t)
                else:
                    xt_t = xt_c0

                if cs > 0:
                    qproj(cs - 1, xt_tiles[cs - 1])

                # KT projection + rope
                kp = kv_ps.tile([128, SC], F32, tag="kt")
                for t in range(DT):
                    nc.tensor.matmul(
                        kp,
                        wk_sb[:, t, :],
                        xt_t[:, t, :],
                        start=(t == 0),
                        stop=(t == DT - 1),
                    )
                rope(kt_sb, kp, cs, "kraw")

                # V^T projection, then transpose 128-subtiles into v_sb
                vp = kv_ps.tile([128, SC], F32, tag="vt")
                for t in range(DT):
                    nc.tensor.matmul(
                        vp,
                        wv_sb[:, t, :],
                        xt_t[:, t, :],
                        start=(t == 0),
                        stop=(t == DT - 1),
                    )
                vt_sb = p1st.tile([128, SC], F32, tag="vtsb", bufs=2)
                nc.vector.tensor_copy(vt_sb, vp)
                for ss in range(SC // 128):
                    sk = cs * (SC // 128) + ss
                    tp = tr_ps.tile([128, 128], F32, tag="tr")
                    nc.tensor.transpose(tp, vt_sb[:, bass.ts(ss, 128)], ident_sb)
                    nc.vector.tensor_copy(v_sb[sk][:, 0:64], tp[:, 0:64])
                    nc.vector.tensor_copy(v_sb[sk][:, 65:129], tp[:, 64:128])
                    nc.gpsimd.tensor_copy(v_sb[sk][:, 64:65], onesc_sb)
                    nc.gpsimd.tensor_copy(v_sb[sk][:, 129:130], onesc_sb)
            qproj(NSC - 1, xt_tiles[NSC - 1])

        # ---------------- Phase 2 + 3: attention + o_proj ------------------------
        with (
            tc.tile_pool(name="wop", bufs=1) as wop,
            tc.tile_pool(name="attnp", bufs=1) as attnp,
            tc.tile_pool(name="expp", bufs=6) as expp,
            tc.tile_pool(name="recp", bufs=4) as recp,
            tc.tile_pool(name="ostg", bufs=3) as ostg,
            tc.tile_pool(name="sc_ps", bufs=2, space="PSUM") as sc_ps,
            tc.tile_pool(name="av_ps", bufs=1, space="PSUM") as av_ps,
            tc.tile_pool(name="mi_ps", bufs=2, space="PSUM") as mi_ps,
        ):
            wo_sb = wop.tile([128, ET, D], BF16, tag="wo")
            nc.gpsimd.dma_start(out=wo_sb, in_=wo.rearrange("(t p) d -> p t d", p=128))
            attn_sb = [attnp.tile([128, S], BF16, tag=f"at{j}", name=f"at{j}") for j in range(NPAIR)]

            pending = []

            def make_normalize(attn_slice, rec, half):
                # rec was computed eagerly at pair end (reciprocal straight
                # from the PSUM denominator row + f32r rounding copy), so the
                # bp matmul popped mid-next-pair does not stall the PE stream.
                def run():
                    bp = mi_ps.tile([128, SC], F32, tag="mi", name="bp")
                    nc.tensor.matmul(bp, ones1_sb, rec, start=True, stop=True)
                    nc.vector.tensor_mul(
                        attn_slice, attn_slice, bp[bass.ds(64 * half, 64), :]
                    )

                return run

            def attention(cs, j, pe_filler=None, last=False):
                """Head pair j (local heads j on kv0, j+4 on kv1), sq chunk cs."""
                sq = bass.ts(cs, SC)
                av_a = av_ps.tile([65, SC], F32, tag="ava")
                av_b = av_ps.tile([65, SC], F32, tag="avb")
                sc_t = [None, None]
                exp_t = [None] * SKT

                def scores(jj):
                    t = sc_ps.tile([128, 2 * SC], F32, tag="sc", name="sc")
                    sc_t[jj % 2] = t
                    nc.tensor.matmul(
                        t[:, 0:SC],
                        kt_sb[0:64, bass.ts(jj, 128)],
                        qt_sb[j][0:64, sq],
                        start=True,
                        stop=True,
                        tile_position=(0, 0),
                    )
                    nc.tensor.matmul(
                        t[:, SC : 2 * SC],
                        kt_sb[64:128, bass.ts(jj, 128)],
                        qt_sb[j][64:128, sq],
                        start=True,
                        stop=True,
                        tile_position=(64, 0),
                    )

                def av(t, start=False, stop=False):
                    nc.tensor.matmul(
                        av_a,
                        v_sb[t][:, 0:65],
                        exp_t[t][:, 0:SC],
                        start=start,
                        stop=stop,
                    )
                    nc.tensor.matmul(
                        av_b,
                        v_sb[t][:, 65:130],
                        exp_t[t][:, SC : 2 * SC],
                        start=start,
                        stop=stop,
                    )

                # AV runs two iterations behind its exp so the PE stream
                # rarely blocks on ACT latency.
                scores(0)
                for jj in range(SKT):
                    et = expp.tile([128, 2 * SC], F32R, tag="exp")
                    exp_t[jj] = et
                    nc.scalar.activation(et, sc_t[jj % 2], AF.Exp, scale=SCALE)
                    if jj + 1 < SKT:
                        scores(jj + 1)
                    if jj >= 2:
                        av(jj - 2, start=(jj == 2))
                    if jj in (4, 6) and pending:
                        pending.pop(0)()
                    if pe_filler is not None:
                        pe_filler(jj)
                av(SKT - 2)
                av(SKT - 1, stop=True)

                # Reciprocals of the denominator rows straight out of PSUM,
                # then raw attn copies (releases av banks); bp broadcast +
                # multiply deferred to early next pair via `pending`.
                # attn copies first (release av banks for the next pair),
                # then den->recip->f32r-rounding chain, all eager on DVE.
                # reciprocal_approx_fast is a bitwise custom-DVE op and CANNOT
                # read PSUM (garbage bits) -- den must bounce through SBUF.
                # On the final pair dens go first: no next pair to unblock,
                # and the drain's bp matmuls wait on the rec chain.
                halves = ((0, av_a), (1, av_b))
                dens = []

                def den_copies():
                    for half, avt in halves:
                        den = recp.tile([1, SC], F32, tag="den", name="den")
                        nc.vector.tensor_copy(den, avt[64:65, :])
                        dens.append(den)

                if last:
                    den_copies()
                for half, avt in halves:
                    attn_slice = attn_sb[j][bass.ds(64 * half, 64), sq]
                    nc.vector.tensor_copy(attn_slice, avt[0:64, :])
                if not last:
                    den_copies()
                for half in (0, 1):
                    rec32 = recp.tile([1, SC], F32, tag="rec32", name="rec32")
                    nc.vector.reciprocal_approx_fast(rec32, dens[half])
                    rec = recp.tile([1, SC], F32R, tag="rec")
                    nc.vector.tensor_copy(rec, rec32)
                    attn_slice = attn_sb[j][bass.ds(64 * half, 64), sq]
                    pending.append(make_normalize(attn_slice, rec, half))

            def make_oproj_filler(cs, st_local):
                """Returns a per-jj filler that emits o_proj work for sq-subtile
                st_local of chunk cs, one dm-chunk (4 mms + copy) at a time,
                spread over the 16-iteration attention j-loop."""
                st = cs * 4 + st_local
                ot = ostg.tile([128, D], F32, tag="ostg", name="ostg")
                state = {"mc": 0}

                def filler(jj):
                    # emit one dm-chunk at jj 5/8/11/14 (after the normalize
                    # pops at jj 3/4, so pair (cs-1, 3) attn is normalized
                    # before the first o_proj read)
                    if jj not in (6, 9, 12, 15) or state["mc"] >= D // SC:
                        return
                    mc = state["mc"]
                    state["mc"] += 1
                    op = mi_ps.tile([128, SC], F32, tag="mi", name="op")
                    for t in range(ET):
                        nc.tensor.matmul(
                            op,
                            attn_sb[t][:, bass.ts(st, 128)],
                            wo_sb[:, t, bass.ts(mc, SC)],
                            start=(t == 0),
                            stop=(t == ET - 1),
                        )
                    nc.vector.tensor_copy(ot[:, bass.ts(mc, SC)], op)
                    nc.sync.dma_start(
                        out=out[bass.ts(st, 128), bass.ts(mc, SC)],
                        in_=ot[:, bass.ts(mc, SC)],
                    )

                return filler

            for cs in range(NSC):
                for j in range(NPAIR):
                    filler = make_oproj_filler(cs - 1, j) if cs > 0 else None
                    attention(cs, j, pe_filler=filler,
                              last=(cs == NSC - 1 and j == NPAIR - 1))
            while pending:
                pending.pop(0)()
            for j in range(NPAIR):
                filler = make_oproj_filler(NSC - 1, j)
                for jj in range(SKT):
                    filler(jj)

    nc.compile()
    return nc


_PROGRAM = None


def _get_program():
    global _PROGRAM
    if _PROGRAM is None:
        _PROGRAM = build_program()
    return _PROGRAM


def _rope_tables():
    inv_freq = 1.0 / (ROPE_BASE ** (np.arange(0, HD, 2, dtype=np.float32) / HD))
    t = np.arange(S, dtype=np.float32)
    freqs = np.outer(t, inv_freq)  # [S, 32]
    emb = np.concatenate([freqs, freqs], axis=-1)  # [S, 64]
    return np.cos(emb).astype(np.float32), np.sin(emb).astype(np.float32)


# Feature permutation within each 64-wide head block: partition p holds
# feature PERM64[p]. Chosen so the RoPE pair (f, f+32) lands 16 partitions
# apart within one 32-partition quadrant (stream_shuffle constraint).
PERM64 = np.array(
    [p if p < 16 else p + 16 if p < 32 else p - 16 if p < 48 else p
     for p in range(64)]
)


def _host_constants():
    cos_t, sin_t = _rope_tables()  # [S, 64]
    idx = PERM64[np.arange(128) % HD]
    # rotate sign for the feature at partition p: rot(q)[f] = -q[f+32] for
    # f%64<32 (else +q[f-32]); with this layout that is p%32 < 16.
    sign = np.where(np.arange(128) % 32 < 16, -1.0, 1.0).astype(np.float32)
    cosd = np.ascontiguousarray(cos_t[:, idx].T)  # [128, S]
    sind = np.ascontiguousarray(sin_t[:, idx].T) * sign[:, None]

    ident = np.eye(128, dtype=np.float32)
    ones1 = np.ones((1, 128), np.float32)
    onesc = np.ones((128, 1), np.float32)
    return cosd, sind, ident, ones1, onesc


def _core_inputs(x, Wq, Wk, Wv, Wo, consts, xt_by_batch, core):
    b, g = divmod(core, 4)
    cosd, sind, ident, ones1, onesc = consts

    wq_c = np.empty((D, E), np.float32)
    wo_c = np.empty((E, D), np.float32)
    for j in range(NPAIR):
        ha = 8 * g + j  # global head, kv-head 2g
        hb = 8 * g + j + 4  # global head, kv-head 2g+1
        # PERM64: q/k feature layout permuted per head (see _host_constants);
        # scores are invariant since q and k use the same permutation.
        wq_c[:, j * 128 : j * 128 + 64] = Wq[:, ha * HD + PERM64]
        wq_c[:, j * 128 + 64 : (j + 1) * 128] = Wq[:, hb * HD + PERM64]
        wo_c[j * 128 : j * 128 + 64, :] = Wo[ha * HD : (ha + 1) * HD, :]
        wo_c[j * 128 + 64 : (j + 1) * 128, :] = Wo[hb * HD : (hb + 1) * HD, :]
    kv0 = 2 * g * HD
    wk_c = np.concatenate(
        [Wk[:, kv0 + PERM64], Wk[:, kv0 + HD + PERM64]], axis=1
    )
    wv_c = np.ascontiguousarray(Wv[:, kv0 : kv0 + KVW])

    bf = ml_dtypes.bfloat16
    return {
        "xt": xt_by_batch[b],
        "wq": wq_c.astype(bf),
        "wk": wk_c.astype(bf),
        "wv": wv_c.astype(bf),
        "wo": wo_c.astype(bf),
        "cosd": cosd,
        "sind": sind,
        "ident": ident,
        "ones1": ones1,
        "onesc": onesc,
    }


def make_in_maps(x, Wq, Wk, Wv, Wo):
    consts = _host_constants()
    xt_by_batch = [
        np.ascontiguousarray(x[b].T).astype(ml_dtypes.bfloat16) for b in range(B)
    ]
    return [
        _core_inputs(x, Wq, Wk, Wv, Wo, consts, xt_by_batch, c)
        for c in range(N_CORES)
    ]


def kernel(x, Wq, Wk, Wv, Wo, _trace=False, _trace_kwargs=None):
    x = np.asarray(x, np.float32)
    Wq = np.asarray(Wq, np.float32)
    Wk = np.asarray(Wk, np.float32)
    Wv = np.asarray(Wv, np.float32)
    Wo = np.asarray(Wo, np.float32)

    nc = _get_program()
    in_maps = make_in_maps(x, Wq, Wk, Wv, Wo)
    res = bass_utils.run_bass_kernel_spmd(
        nc,
        in_maps,
        core_ids=list(range(N_CORES)),
        trace=_trace,
        **(_trace_kwargs or {}),
    )
    outs = [r["out"] for r in res.results]
    full = np.empty((B, S, D), np.float32)
    for b in range(B):
        full[b] = outs[4 * b] + outs[4 * b + 1] + outs[4 * b + 2] + outs[4 * b + 3]
    if _trace:
        return full, res
    return full



# revision 28
# speedup vs baseline: 1.1814x; 1.1814x over previous
"""GQA attention block (Wq/Wk/Wv -> RoPE -> softmax(QK^T)V -> Wo) on 8 Trainium2
NeuronCores.

Sharding (tensor-parallel per the head-sharding scheme):
  core c in 0..7: batch b = c // 4, head-group g = c % 4.
  Each core owns 8 q-heads (global 8g..8g+7) and 2 kv-heads (2g, 2g+1) of one
  batch element, computes its slice of q/k/v projections, RoPE, attention, and
  a partial o_proj (Wo rows for its heads). The all-reduce after o_proj is the
  host-side unshard: out[b] = sum of the 4 partial outputs of batch b.

On-device layout (per core), everything feature-on-partitions ("transposed"):
  xt    [D=2048, S=2048]   x^T for this batch
  QT    [E=512,  S]        q^T; partition-tile j holds head pair (j, j+4):
                           local head j (kv0) on partitions 0:64, head j+4
                           (kv1) on partitions 64:128. Wq columns are permuted
                           on the host to produce this layout directly.
  KT    [128, S]           k^T; kv0 on partitions 0:64, kv1 on 64:128.
  V     [S, 130] as 16 tiles [128, 130]: cols 0:64 v(kv0), col 64 ones,
                           cols 65:129 v(kv1), col 129 ones  (v_aug).
  scores^T per head: [sk, sq] so exp is ACT psum->sbuf and the attn@v
  contraction (over sk) uses v_aug as the stationary operand; row 64 of the
  attn@v output is the softmax denominator (ones column trick).

Matmuls run as float32r (full-rate fp32 on the PE; ~1.6e-4 rel err measured).
"""

import sys

if "/opt/trn_rl_repo" not in sys.path:
    sys.path.insert(0, "/opt/trn_rl_repo")

from contextlib import ExitStack

import numpy as np
import ml_dtypes

import concourse.bass as bass  # noqa: F401  (engine types via nc)
import concourse.tile as tile
from concourse import bacc, bass_utils, mybir

F32 = mybir.dt.float32
F32R = mybir.dt.float32r
BF16 = mybir.dt.bfloat16
AF = mybir.ActivationFunctionType

# Problem constants (hardcoded per harness contract)
B = 2
S = 2048  # sequence length
D = 2048  # d_model
N_HEADS = 32
N_KV = 8
HD = 64  # head dim
ROPE_BASE = 500000.0
N_CORES = 8

# Per-core derived
NQ = N_HEADS // 4  # 8 local q heads (4 head-groups)
E = NQ * HD  # 512 local q features
NPAIR = NQ // 2  # 4 head pairs / e-tiles
KVW = 2 * HD  # 128 local kv features
SC = 512  # s-chunk (projection + sq chunk)
NSC = S // SC  # 4
DT = D // 128  # 16 d-tiles
SKT = S // 128  # 16 sk tiles
ET = E // 128  # 4 e-tiles
SCALE = 1.0 / float(np.sqrt(HD))


def build_program():
    nc = bacc.Bacc(
        "TRN2", target_bir_lowering=False, debug=False, enable_asserts=False
    )

    xt = nc.dram_tensor("xt", [D, S], BF16, kind="ExternalInput").ap()
    wq = nc.dram_tensor("wq", [D, E], BF16, kind="ExternalInput").ap()
    wk = nc.dram_tensor("wk", [D, KVW], BF16, kind="ExternalInput").ap()
    wv = nc.dram_tensor("wv", [D, KVW], BF16, kind="ExternalInput").ap()
    wo = nc.dram_tensor("wo", [E, D], BF16, kind="ExternalInput").ap()
    cosd = nc.dram_tensor("cosd", [128, S], F32R, kind="ExternalInput").ap()
    sind = nc.dram_tensor("sind", [128, S], F32R, kind="ExternalInput").ap()
    ident = nc.dram_tensor("ident", [128, 128], F32, kind="ExternalInput").ap()
    ones1 = nc.dram_tensor("ones1", [1, 128], F32R, kind="ExternalInput").ap()
    onesc = nc.dram_tensor("onesc", [128, 1], F32R, kind="ExternalInput").ap()
    out = nc.dram_tensor("out", [S, D], F32, kind="ExternalOutput").ap()

    with tile.TileContext(nc) as tc, ExitStack() as ctx:
        persist = ctx.enter_context(tc.tile_pool(name="persist", bufs=1))

        # Persistent SBUF state
        qt_sb = [persist.tile([128, S], F32R, tag=f"qt{j}", name=f"qt{j}") for j in range(NPAIR)]
        kt_sb = persist.tile([128, S], F32R, tag="kt")
        v_sb = [persist.tile([128, 130], F32R, tag=f"v{j}", name=f"v{j}") for j in range(SKT)]
        onesc_sb = persist.tile([128, 1], F32R, tag="onesc")
        nc.sync.dma_start(out=onesc_sb, in_=onesc)
        ones1_sb = persist.tile([1, 128], F32R, tag="ones1")
        nc.sync.dma_start(out=ones1_sb, in_=ones1)

        # ---------------- Phase 1: projections + RoPE + V transpose -------------
        with (
            tc.tile_pool(name="xtp", bufs=2) as xtp,
            tc.tile_pool(name="wp", bufs=1) as wp,
            tc.tile_pool(name="ropec", bufs=1) as ropec,
            tc.tile_pool(name="p1st", bufs=3) as p1st,
            tc.tile_pool(name="qt_ps", bufs=2, space="PSUM") as qt_ps,
            tc.tile_pool(name="kv_ps", bufs=1, space="PSUM") as kv_ps,
            tc.tile_pool(name="tr_ps", bufs=2, space="PSUM") as tr_ps,
        ):

            wq_r = wq.rearrange("(t p) e -> p t e", p=128)
            wk_r = wk.rearrange("(t p) e -> p t e", p=128)
            wv_r = wv.rearrange("(t p) e -> p t e", p=128)
            wq_sb = wp.tile([128, DT, E], BF16, tag="wq")
            wk_sb = wp.tile([128, DT, KVW], BF16, tag="wk")
            wv_sb = wp.tile([128, DT, KVW], BF16, tag="wv")
            xt_r = xt.rearrange("(t p) s -> p t s", p=128)
            xt_c0 = xtp.tile([128, DT, SC], BF16, tag="xt", name="xt_c0")
            # wq via gpsimd SWDGE immediately (own queue): Q-proj of chunk 0
            # starts ~12us in and must not queue behind cos/sin on the ACT ring.
            nc.gpsimd.dma_start(out=wq_sb, in_=wq_r)
            nc.gpsimd.dma_start(out=wv_sb, in_=wv_r)
            ident_sb = ropec.tile([128, 128], F32, tag="ident")
            nc.scalar.dma_start(out=ident_sb, in_=ident)
            cos_sb = ropec.tile([128, S], F32R, tag="cos")
            nc.scalar.dma_start(out=cos_sb, in_=cosd)
            sin_sb = ropec.tile([128, S], F32R, tag="sin")
            nc.scalar.dma_start(out=sin_sb, in_=sind)
            # xt+wk interleaved per-tile: K-projection of chunk 0 unblocks
            # fastest.
            for t in range(DT):
                nc.sync.dma_start(out=xt_c0[:, t, :], in_=xt_r[:, t, 0:SC])
                nc.sync.dma_start(out=wk_sb[:, t, :], in_=wk_r[:, t, :])


            SHUF_MASK = [(i + 16) % 32 for i in range(32)]

            def rope(dst, src_ps, cs, raw_tag):
                """dst[:, cs*SC:+SC] = src_ps*cos + shuffle(src)*sin_signed.

                Features are laid out (host-side permutation) so the RoPE
                rotate pairing is a +-16 swap within each 32-partition
                quadrant; the rotate sign is folded into sind."""
                sl = bass.ts(cs, SC)
                raw = p1st.tile([128, SC], F32R, tag="raw", name="raw", bufs=3)
                nc.scalar.copy(raw, src_ps)
                rp = p1st.tile([128, SC], F32, tag="shuf", name="shuf", bufs=3)
                nc.vector.stream_shuffle(rp, raw, SHUF_MASK)
                tcos = p1st.tile([128, SC], F32, tag="tmp", name="tcos", bufs=4)
                nc.vector.tensor_mul(tcos, raw, cos_sb[:, sl])
                tsin = p1st.tile([128, SC], F32, tag="tmp", name="tsin", bufs=4)
                nc.vector.tensor_mul(tsin, rp, sin_sb[:, sl])
                nc.vector.tensor_add(dst[:, sl], tcos, tsin)

            for cs in range(NSC):
                if cs == 0:
                    xt_t = xt_c0
                else:
                    # single batched DMA per chunk: latency hidden behind
                    # chunk cs-1 compute, 1/16th the SP sequencer issue cost
                    xt_t = xtp.tile([128, DT, SC], BF16, tag="xt")
                    nc.sync.dma_start(
                        out=xt_t, in_=xt_r[:, :, bass.ts(cs, SC)]
                    )

                # KT projection + rope
                kp = kv_ps.tile([128, SC], F32, tag="kt")
                for t in range(DT):
                    nc.tensor.matmul(
                        kp,
                        wk_sb[:, t, :],
                        xt_t[:, t, :],
                        start=(t == 0),
                        stop=(t == DT - 1),
                    )
                rope(kt_sb, kp, cs, "kraw")

                # V^T projection, then transpose 128-subtiles into v_sb
                vp = kv_ps.tile([128, SC], F32, tag="vt")
                for t in range(DT):
                    nc.tensor.matmul(
                        vp,
                        wv_sb[:, t, :],
                        xt_t[:, t, :],
                        start=(t == 0),
                        stop=(t == DT - 1),
                    )
                vt_sb = p1st.tile([128, SC], F32, tag="vtsb", bufs=2)
                nc.vector.tensor_copy(vt_sb, vp)
                for ss in range(SC // 128):
                    sk = cs * (SC // 128) + ss
                    tp = tr_ps.tile([128, 128], F32, tag="tr")
                    nc.tensor.transpose(tp, vt_sb[:, bass.ts(ss, 128)], ident_sb)
                    nc.vector.tensor_copy(v_sb[sk][:, 0:64], tp[:, 0:64])
                    nc.vector.tensor_copy(v_sb[sk][:, 65:129], tp[:, 64:128])
                    nc.gpsimd.tensor_copy(v_sb[sk][:, 64:65], onesc_sb)
                    nc.gpsimd.tensor_copy(v_sb[sk][:, 129:130], onesc_sb)

                # QT projection + rope, per e-tile (head pair)
                for j in range(NPAIR):
                    qp = qt_ps.tile([128, SC], F32, tag="qt")
                    for t in range(DT):
                        nc.tensor.matmul(
                            qp,
                            wq_sb[:, t, bass.ts(j, 128)],
                            xt_t[:, t, :],
                            start=(t == 0),
                            stop=(t == DT - 1),
                        )
                    rope(qt_sb[j], qp, cs, "qraw")

        # ---------------- Phase 2 + 3: attention + o_proj ------------------------
        with (
            tc.tile_pool(name="wop", bufs=1) as wop,
            tc.tile_pool(name="attnp", bufs=1) as attnp,
            tc.tile_pool(name="expp", bufs=6) as expp,
            tc.tile_pool(name="recp", bufs=4) as recp,
            tc.tile_pool(name="ostg", bufs=3) as ostg,
            tc.tile_pool(name="sc_ps", bufs=2, space="PSUM") as sc_ps,
            tc.tile_pool(name="av_ps", bufs=1, space="PSUM") as av_ps,
            tc.tile_pool(name="mi_ps", bufs=2, space="PSUM") as mi_ps,
        ):
            wo_sb = wop.tile([128, ET, D], BF16, tag="wo")
            nc.gpsimd.dma_start(out=wo_sb, in_=wo.rearrange("(t p) d -> p t d", p=128))
            attn_sb = [attnp.tile([128, S], BF16, tag=f"at{j}", name=f"at{j}") for j in range(NPAIR)]

            pending = []

            def make_normalize(attn_slice, rec, half):
                # rec was computed eagerly at pair end (reciprocal straight
                # from the PSUM denominator row + f32r rounding copy), so the
                # bp matmul popped mid-next-pair does not stall the PE stream.
                def run():
                    bp = mi_ps.tile([128, SC], F32, tag="mi", name="bp")
                    nc.tensor.matmul(bp, ones1_sb, rec, start=True, stop=True)
                    nc.vector.tensor_mul(
                        attn_slice, attn_slice, bp[bass.ds(64 * half, 64), :]
                    )

                return run

            def attention(cs, j, pe_filler=None, last=False):
                """Head pair j (local heads j on kv0, j+4 on kv1), sq chunk cs."""
                sq = bass.ts(cs, SC)
                av_a = av_ps.tile([65, SC], F32, tag="ava")
                av_b = av_ps.tile([65, SC], F32, tag="avb")
                sc_t = [None, None]
                exp_t = [None] * SKT

                def scores(jj):
                    t = sc_ps.tile([128, 2 * SC], F32, tag="sc", name="sc")
                    sc_t[jj % 2] = t
                    nc.tensor.matmul(
                        t[:, 0:SC],
                        kt_sb[0:64, bass.ts(jj, 128)],
                        qt_sb[j][0:64, sq],
                        start=True,
                        stop=True,
                        tile_position=(0, 0),
                    )
                    nc.tensor.matmul(
                        t[:, SC : 2 * SC],
                        kt_sb[64:128, bass.ts(jj, 128)],
                        qt_sb[j][64:128, sq],
                        start=True,
                        stop=True,
                        tile_position=(64, 0),
                    )

                def av(t, start=False, stop=False):
                    nc.tensor.matmul(
                        av_a,
                        v_sb[t][:, 0:65],
                        exp_t[t][:, 0:SC],
                        start=start,
                        stop=stop,
                    )
                    nc.tensor.matmul(
                        av_b,
                        v_sb[t][:, 65:130],
                        exp_t[t][:, SC : 2 * SC],
                        start=start,
                        stop=stop,
                    )

                # AV runs two iterations behind its exp so the PE stream
                # rarely blocks on ACT latency.
                scores(0)
                for jj in range(SKT):
                    et = expp.tile([128, 2 * SC], F32R, tag="exp")
                    exp_t[jj] = et
                    nc.scalar.activation(et, sc_t[jj % 2], AF.Exp, scale=SCALE)
                    if jj + 1 < SKT:
                        scores(jj + 1)
                    if jj >= 2:
                        av(jj - 2, start=(jj == 2))
                    if jj in (4, 6) and pending:
                        pending.pop(0)()
                    if pe_filler is not None:
                        pe_filler(jj)
                av(SKT - 2)
                av(SKT - 1, stop=True)

                # Reciprocals of the denominator rows straight out of PSUM,
                # then raw attn copies (releases av banks); bp broadcast +
                # multiply deferred to early next pair via `pending`.
                # attn copies first (release av banks for the next pair),
                # then den->recip->f32r-rounding chain, all eager on DVE.
                # reciprocal_approx_fast is a bitwise custom-DVE op and CANNOT
                # read PSUM (garbage bits) -- den must bounce through SBUF.
                # On the final pair dens go first: no next pair to unblock,
                # and the drain's bp matmuls wait on the rec chain.
                halves = ((0, av_a), (1, av_b))
                dens = []

                def den_copies():
                    for half, avt in halves:
                        den = recp.tile([1, SC], F32, tag="den", name="den")
                        nc.vector.tensor_copy(den, avt[64:65, :])
                        dens.append(den)

                if last:
                    den_copies()
                for half, avt in halves:
                    attn_slice = attn_sb[j][bass.ds(64 * half, 64), sq]
                    nc.vector.tensor_copy(attn_slice, avt[0:64, :])
                if not last:
                    den_copies()
                for half in (0, 1):
                    rec32 = recp.tile([1, SC], F32, tag="rec32", name="rec32")
                    nc.vector.reciprocal_approx_fast(rec32, dens[half])
                    rec = recp.tile([1, SC], F32R, tag="rec")
                    nc.vector.tensor_copy(rec, rec32)
                    attn_slice = attn_sb[j][bass.ds(64 * half, 64), sq]
                    pending.append(make_normalize(attn_slice, rec, half))

            def make_oproj_filler(cs, st_local):
                """Returns a per-jj filler that emits o_proj work for sq-subtile
                st_local of chunk cs, one dm-chunk (4 mms + copy) at a time,
                spread over the 16-iteration attention j-loop."""
                st = cs * 4 + st_local
                ot = ostg.tile([128, D], F32, tag="ostg", name="ostg")
                state = {"mc": 0}

                def filler(jj):
                    # emit one dm-chunk at jj 5/8/11/14 (after the normalize
                    # pops at jj 3/4, so pair (cs-1, 3) attn is normalized
                    # before the first o_proj read)
                    if jj not in (6, 9, 12, 15) or state["mc"] >= D // SC:
                        return
                    mc = state["mc"]
                    state["mc"] += 1
                    op = mi_ps.tile([128, SC], F32, tag="mi", name="op")
                    for t in range(ET):
                        nc.tensor.matmul(
                            op,
                            attn_sb[t][:, bass.ts(st, 128)],
                            wo_sb[:, t, bass.ts(mc, SC)],
                            start=(t == 0),
                            stop=(t == ET - 1),
                        )
                    nc.vector.tensor_copy(ot[:, bass.ts(mc, SC)], op)
                    nc.sync.dma_start(
                        out=out[bass.ts(st, 128), bass.ts(mc, SC)],
                        in_=ot[:, bass.ts(mc, SC)],
                    )

                return filler

            for cs in range(NSC):
                for j in range(NPAIR):
                    filler = make_oproj_filler(cs - 1, j) if cs > 0 else None
                    attention(cs, j, pe_filler=filler,
                              last=(cs == NSC - 1 and j == NPAIR - 1))
            while pending:
                pending.pop(0)()
            for j in range(NPAIR):
                filler = make_oproj_filler(NSC - 1, j)
                for jj in range(SKT):
                    filler(jj)

    nc.compile()
    return nc


_PROGRAM = None


def _get_program():
    global _PROGRAM
    if _PROGRAM is None:
        _PROGRAM = build_program()
    return _PROGRAM


def _rope_tables():
    inv_freq = 1.0 / (ROPE_BASE ** (np.arange(0, HD, 2, dtype=np.float32) / HD))
    t = np.arange(S, dtype=np.float32)
    freqs = np.outer(t, inv_freq)  # [S, 32]
    emb = np.concatenate([freqs, freqs], axis=-1)  # [S, 64]
    return np.cos(emb).astype(np.float32), np.sin(emb).astype(np.float32)


# Feature permutation within each 64-wide head block: partition p holds
# feature PERM64[p]. Chosen so the RoPE pair (f, f+32) lands 16 partitions
# apart within one 32-partition quadrant (stream_shuffle constraint).
PERM64 = np.array(
    [p if p < 16 else p + 16 if p < 32 else p - 16 if p < 48 else p
     for p in range(64)]
)


def _host_constants():
    cos_t, sin_t = _rope_tables()  # [S, 64]
    idx = PERM64[np.arange(128) % HD]
    # rotate sign for the feature at partition p: rot(q)[f] = -q[f+32] for
    # f%64<32 (else +q[f-32]); with this layout that is p%32 < 16.
    sign = np.where(np.arange(128) % 32 < 16, -1.0, 1.0).astype(np.float32)
    cosd = np.ascontiguousarray(cos_t[:, idx].T)  # [128, S]
    sind = np.ascontiguousarray(sin_t[:, idx].T) * sign[:, None]

    ident = np.eye(128, dtype=np.float32)
    ones1 = np.ones((1, 128), np.float32)
    onesc = np.ones((128, 1), np.float32)
    return cosd, sind, ident, ones1, onesc


def _core_inputs(x, Wq, Wk, Wv, Wo, consts, xt_by_batch, core):
    b, g = divmod(core, 4)
    cosd, sind, ident, ones1, onesc = consts

    wq_c = np.empty((D, E), np.float32)
    wo_c = np.empty((E, D), np.float32)
    for j in range(NPAIR):
        ha = 8 * g + j  # global head, kv-head 2g
        hb = 8 * g + j + 4  # global head, kv-head 2g+1
        # PERM64: q/k feature layout permuted per head (see _host_constants);
        # scores are invariant since q and k use the same permutation.
        wq_c[:, j * 128 : j * 128 + 64] = Wq[:, ha * HD + PERM64]
        wq_c[:, j * 128 + 64 : (j + 1) * 128] = Wq[:, hb * HD + PERM64]
        wo_c[j * 128 : j * 128 + 64, :] = Wo[ha * HD : (ha + 1) * HD, :]
        wo_c[j * 128 + 64 : (j + 1) * 128, :] = Wo[hb * HD : (hb + 1) * HD, :]
    kv0 = 2 * g * HD
    wk_c = np.concatenate(
        [Wk[:, kv0 + PERM64], Wk[:, kv0 + HD + PERM64]], axis=1
    )
    wv_c = np.ascontiguousarray(Wv[:, kv0 : kv0 + KVW])

    bf = ml_dtypes.bfloat16
    return {
        "xt": xt_by_batch[b],
        "wq": wq_c.astype(bf),
        "wk": wk_c.astype(bf),
        "wv": wv_c.astype(bf),
        "wo": wo_c.astype(bf),
        "cosd": cosd,
        "sind": sind,
        "ident": ident,
        "ones1": ones1,
        "onesc": onesc,
    }


def make_in_maps(x, Wq, Wk, Wv, Wo):
    consts = _host_constants()
    xt_by_batch = [
        np.ascontiguousarray(x[b].T).astype(ml_dtypes.bfloat16) for b in range(B)
    ]
    return [
        _core_inputs(x, Wq, Wk, Wv, Wo, consts, xt_by_batch, c)
        for c in range(N_CORES)
    ]


def kernel(x, Wq, Wk, Wv, Wo, _trace=False, _trace_kwargs=None):
    x = np.asarray(x, np.float32)
    Wq = np.asarray(Wq, np.float32)
    Wk = np.asarray(Wk, np.float32)
    Wv = np.asarray(Wv, np.float32)
    Wo = np.asarray(Wo, np.float32)

    nc = _get_program()
    in_maps = make_in_maps(x, Wq, Wk, Wv, Wo)
    res = bass_utils.run_bass_kernel_spmd(
        nc,
        in_maps,
        core_ids=list(range(N_CORES)),
        trace=_trace,
        **(_trace_kwargs or {}),
    )
    outs = [r["out"] for r in res.results]
    full = np.empty((B, S, D), np.float32)
    for b in range(B):
        full[b] = outs[4 * b] + outs[4 * b + 1] + outs[4 * b + 2] + outs[4 * b + 3]
    if _trace:
        return full, res
    return full



# revision 29
# speedup vs baseline: 1.3128x; 1.1113x over previous
"""GQA attention block (Wq/Wk/Wv -> RoPE -> softmax(QK^T)V -> Wo) on 8 Trainium2
NeuronCores.

Sharding (tensor-parallel per the head-sharding scheme):
  core c in 0..7: batch b = c // 4, head-group g = c % 4.
  Each core owns 8 q-heads (global 8g..8g+7) and 2 kv-heads (2g, 2g+1) of one
  batch element, computes its slice of q/k/v projections, RoPE, attention, and
  a partial o_proj (Wo rows for its heads). The all-reduce after o_proj is the
  host-side unshard: out[b] = sum of the 4 partial outputs of batch b.

On-device layout (per core), everything feature-on-partitions ("transposed"):
  xt    [D=2048, S=2048]   x^T for this batch
  QT    [E=512,  S]        q^T; partition-tile j holds head pair (j, j+4):
                           local head j (kv0) on partitions 0:64, head j+4
                           (kv1) on partitions 64:128. Wq columns are permuted
                           on the host to produce this layout directly.
  KT    [128, S]           k^T; kv0 on partitions 0:64, kv1 on 64:128.
  V     [S, 130] as 16 tiles [128, 130]: cols 0:64 v(kv0), col 64 ones,
                           cols 65:129 v(kv1), col 129 ones  (v_aug).
  scores^T per head: [sk, sq] so exp is ACT psum->sbuf and the attn@v
  contraction (over sk) uses v_aug as the stationary operand; row 64 of the
  attn@v output is the softmax denominator (ones column trick).

Matmuls run as float32r (full-rate fp32 on the PE; ~1.6e-4 rel err measured).
"""

import sys

if "/opt/trn_rl_repo" not in sys.path:
    sys.path.insert(0, "/opt/trn_rl_repo")

from contextlib import ExitStack

import numpy as np
import ml_dtypes

import concourse.bass as bass  # noqa: F401  (engine types via nc)
import concourse.tile as tile
from concourse import bacc, bass_utils, mybir

F32 = mybir.dt.float32
F32R = mybir.dt.float32r
BF16 = mybir.dt.bfloat16
AF = mybir.ActivationFunctionType

# Problem constants (hardcoded per harness contract)
B = 2
S = 2048  # sequence length
D = 2048  # d_model
N_HEADS = 32
N_KV = 8
HD = 64  # head dim
ROPE_BASE = 500000.0
N_CORES = 8

# Per-core derived
NQ = N_HEADS // 4  # 8 local q heads (4 head-groups)
E = NQ * HD  # 512 local q features
NPAIR = NQ // 2  # 4 head pairs / e-tiles
KVW = 2 * HD  # 128 local kv features
SC = 512  # s-chunk (projection + sq chunk)
NSC = S // SC  # 4
DT = D // 128  # 16 d-tiles
SKT = S // 128  # 16 sk tiles
ET = E // 128  # 4 e-tiles
SCALE = 1.0 / float(np.sqrt(HD))


def build_program():
    nc = bacc.Bacc(
        "TRN2", target_bir_lowering=False, debug=False, enable_asserts=False
    )

    xt = nc.dram_tensor("xt", [D, S], BF16, kind="ExternalInput").ap()
    wq = nc.dram_tensor("wq", [D, E], BF16, kind="ExternalInput").ap()
    wk = nc.dram_tensor("wk", [D, KVW], BF16, kind="ExternalInput").ap()
    wv = nc.dram_tensor("wv", [D, KVW], BF16, kind="ExternalInput").ap()
    wo = nc.dram_tensor("wo", [E, D], BF16, kind="ExternalInput").ap()
    cosd = nc.dram_tensor("cosd", [128, S], F32R, kind="ExternalInput").ap()
    sind = nc.dram_tensor("sind", [128, S], F32R, kind="ExternalInput").ap()
    ident = nc.dram_tensor("ident", [128, 128], F32, kind="ExternalInput").ap()
    ones1 = nc.dram_tensor("ones1", [1, 128], F32R, kind="ExternalInput").ap()
    onesc = nc.dram_tensor("onesc", [128, 1], BF16, kind="ExternalInput").ap()
    out = nc.dram_tensor("out", [S, D], F32, kind="ExternalOutput").ap()

    with tile.TileContext(nc) as tc, ExitStack() as ctx:
        persist = ctx.enter_context(tc.tile_pool(name="persist", bufs=1))

        # Persistent SBUF state
        qt_sb = [persist.tile([128, S], BF16, tag=f"qt{j}", name=f"qt{j}") for j in range(NPAIR)]
        kt_sb = persist.tile([128, S], BF16, tag="kt")
        v_sb = [persist.tile([128, 130], BF16, tag=f"v{j}", name=f"v{j}") for j in range(SKT)]
        onesc_sb = persist.tile([128, 1], BF16, tag="onesc")
        nc.sync.dma_start(out=onesc_sb, in_=onesc)
        ones1_sb = persist.tile([1, 128], F32R, tag="ones1")
        nc.sync.dma_start(out=ones1_sb, in_=ones1)

        # ---------------- Phase 1: projections + RoPE + V transpose -------------
        with (
            tc.tile_pool(name="xtp", bufs=2) as xtp,
            tc.tile_pool(name="wp", bufs=1) as wp,
            tc.tile_pool(name="ropec", bufs=1) as ropec,
            tc.tile_pool(name="p1st", bufs=3) as p1st,
            tc.tile_pool(name="qt_ps", bufs=2, space="PSUM") as qt_ps,
            tc.tile_pool(name="kv_ps", bufs=1, space="PSUM") as kv_ps,
            tc.tile_pool(name="tr_ps", bufs=2, space="PSUM") as tr_ps,
        ):

            wq_r = wq.rearrange("(t p) e -> p t e", p=128)
            wk_r = wk.rearrange("(t p) e -> p t e", p=128)
            wv_r = wv.rearrange("(t p) e -> p t e", p=128)
            wq_sb = wp.tile([128, DT, E], BF16, tag="wq")
            wk_sb = wp.tile([128, DT, KVW], BF16, tag="wk")
            wv_sb = wp.tile([128, DT, KVW], BF16, tag="wv")
            xt_r = xt.rearrange("(t p) s -> p t s", p=128)
            xt_c0 = xtp.tile([128, DT, SC], BF16, tag="xt", name="xt_c0")
            # wq via gpsimd SWDGE immediately (own queue): Q-proj of chunk 0
            # starts ~12us in and must not queue behind cos/sin on the ACT ring.
            nc.gpsimd.dma_start(out=wq_sb, in_=wq_r)
            nc.gpsimd.dma_start(out=wv_sb, in_=wv_r)
            ident_sb = ropec.tile([128, 128], F32, tag="ident")
            nc.scalar.dma_start(out=ident_sb, in_=ident)
            cos_sb = ropec.tile([128, S], F32R, tag="cos")
            nc.scalar.dma_start(out=cos_sb, in_=cosd)
            sin_sb = ropec.tile([128, S], F32R, tag="sin")
            nc.scalar.dma_start(out=sin_sb, in_=sind)
            # xt+wk interleaved per-tile: K-projection of chunk 0 unblocks
            # fastest.
            for t in range(DT):
                nc.sync.dma_start(out=xt_c0[:, t, :], in_=xt_r[:, t, 0:SC])
                nc.sync.dma_start(out=wk_sb[:, t, :], in_=wk_r[:, t, :])


            SHUF_MASK = [(i + 16) % 32 for i in range(32)]

            def rope(dst, src_ps, cs, raw_tag):
                """dst[:, cs*SC:+SC] = src_ps*cos + shuffle(src)*sin_signed.

                Features are laid out (host-side permutation) so the RoPE
                rotate pairing is a +-16 swap within each 32-partition
                quadrant; the rotate sign is folded into sind."""
                sl = bass.ts(cs, SC)
                raw = p1st.tile([128, SC], F32R, tag="raw", name="raw", bufs=3)
                nc.scalar.copy(raw, src_ps)
                rp = p1st.tile([128, SC], F32, tag="shuf", name="shuf", bufs=3)
                nc.vector.stream_shuffle(rp, raw, SHUF_MASK)
                tcos = p1st.tile([128, SC], F32, tag="tmp", name="tcos", bufs=4)
                nc.vector.tensor_mul(tcos, raw, cos_sb[:, sl])
                tsin = p1st.tile([128, SC], F32, tag="tmp", name="tsin", bufs=4)
                nc.vector.tensor_mul(tsin, rp, sin_sb[:, sl])
                nc.vector.tensor_add(dst[:, sl], tcos, tsin)

            for cs in range(NSC):
                if cs == 0:
                    xt_t = xt_c0
                else:
                    # single batched DMA per chunk: latency hidden behind
                    # chunk cs-1 compute, 1/16th the SP sequencer issue cost
                    xt_t = xtp.tile([128, DT, SC], BF16, tag="xt")
                    nc.sync.dma_start(
                        out=xt_t, in_=xt_r[:, :, bass.ts(cs, SC)]
                    )

                # KT projection + rope
                kp = kv_ps.tile([128, SC], F32, tag="kt")
                for t in range(DT):
                    nc.tensor.matmul(
                        kp,
                        wk_sb[:, t, :],
                        xt_t[:, t, :],
                        start=(t == 0),
                        stop=(t == DT - 1),
                    )
                rope(kt_sb, kp, cs, "kraw")

                # V^T projection, then transpose 128-subtiles into v_sb
                vp = kv_ps.tile([128, SC], F32, tag="vt")
                for t in range(DT):
                    nc.tensor.matmul(
                        vp,
                        wv_sb[:, t, :],
                        xt_t[:, t, :],
                        start=(t == 0),
                        stop=(t == DT - 1),
                    )
                vt_sb = p1st.tile([128, SC], F32, tag="vtsb", bufs=2)
                nc.vector.tensor_copy(vt_sb, vp)
                for ss in range(SC // 128):
                    sk = cs * (SC // 128) + ss
                    tp = tr_ps.tile([128, 128], F32, tag="tr")
                    nc.tensor.transpose(tp, vt_sb[:, bass.ts(ss, 128)], ident_sb)
                    nc.vector.tensor_copy(v_sb[sk][:, 0:64], tp[:, 0:64])
                    nc.vector.tensor_copy(v_sb[sk][:, 65:129], tp[:, 64:128])
                    nc.gpsimd.tensor_copy(v_sb[sk][:, 64:65], onesc_sb)
                    nc.gpsimd.tensor_copy(v_sb[sk][:, 129:130], onesc_sb)

                # QT projection + rope, per e-tile (head pair)
                for j in range(NPAIR):
                    qp = qt_ps.tile([128, SC], F32, tag="qt")
                    for t in range(DT):
                        nc.tensor.matmul(
                            qp,
                            wq_sb[:, t, bass.ts(j, 128)],
                            xt_t[:, t, :],
                            start=(t == 0),
                            stop=(t == DT - 1),
                        )
                    rope(qt_sb[j], qp, cs, "qraw")

        # ---------------- Phase 2 + 3: attention + o_proj ------------------------
        with (
            tc.tile_pool(name="wop", bufs=1) as wop,
            tc.tile_pool(name="attnp", bufs=1) as attnp,
            tc.tile_pool(name="expp", bufs=6) as expp,
            tc.tile_pool(name="recp", bufs=4) as recp,
            tc.tile_pool(name="ostg", bufs=3) as ostg,
            tc.tile_pool(name="sc_ps", bufs=2, space="PSUM") as sc_ps,
            tc.tile_pool(name="av_ps", bufs=1, space="PSUM") as av_ps,
            tc.tile_pool(name="mi_ps", bufs=2, space="PSUM") as mi_ps,
        ):
            wo_sb = wop.tile([128, ET, D], BF16, tag="wo")
            nc.gpsimd.dma_start(out=wo_sb, in_=wo.rearrange("(t p) d -> p t d", p=128))
            attn_sb = [attnp.tile([128, S], BF16, tag=f"at{j}", name=f"at{j}") for j in range(NPAIR)]

            pending = []

            def make_normalize(attn_slice, rec, half):
                # rec was computed eagerly at pair end (reciprocal straight
                # from the PSUM denominator row + f32r rounding copy), so the
                # bp matmul popped mid-next-pair does not stall the PE stream.
                def run():
                    bp = mi_ps.tile([128, SC], F32, tag="mi", name="bp")
                    nc.tensor.matmul(bp, ones1_sb, rec, start=True, stop=True)
                    nc.vector.tensor_mul(
                        attn_slice, attn_slice, bp[bass.ds(64 * half, 64), :]
                    )

                return run

            def attention(cs, j, pe_filler=None, last=False):
                """Head pair j (local heads j on kv0, j+4 on kv1), sq chunk cs."""
                sq = bass.ts(cs, SC)
                av_a = av_ps.tile([65, SC], F32, tag="ava")
                av_b = av_ps.tile([65, SC], F32, tag="avb")
                sc_t = [None, None]
                exp_t = [None] * SKT

                def scores(jj):
                    t = sc_ps.tile([128, 2 * SC], F32, tag="sc", name="sc")
                    sc_t[jj % 2] = t
                    nc.tensor.matmul(
                        t[:, 0:SC],
                        kt_sb[0:64, bass.ts(jj, 128)],
                        qt_sb[j][0:64, sq],
                        start=True,
                        stop=True,
                        tile_position=(0, 0),
                    )
                    nc.tensor.matmul(
                        t[:, SC : 2 * SC],
                        kt_sb[64:128, bass.ts(jj, 128)],
                        qt_sb[j][64:128, sq],
                        start=True,
                        stop=True,
                        tile_position=(64, 0),
                    )

                def av(t, start=False, stop=False):
                    nc.tensor.matmul(
                        av_a,
                        v_sb[t][:, 0:65],
                        exp_t[t][:, 0:SC],
                        start=start,
                        stop=stop,
                    )
                    nc.tensor.matmul(
                        av_b,
                        v_sb[t][:, 65:130],
                        exp_t[t][:, SC : 2 * SC],
                        start=start,
                        stop=stop,
                    )

                # AV runs two iterations behind its exp so the PE stream
                # rarely blocks on ACT latency.
                scores(0)
                for jj in range(SKT):
                    et = expp.tile([128, 2 * SC], BF16, tag="exp")
                    exp_t[jj] = et
                    nc.scalar.activation(et, sc_t[jj % 2], AF.Exp, scale=SCALE)
                    if jj + 1 < SKT:
                        scores(jj + 1)
                    if jj >= 2:
                        av(jj - 2, start=(jj == 2))
                    if jj in (4, 6) and pending:
                        pending.pop(0)()
                    if pe_filler is not None:
                        pe_filler(jj)
                av(SKT - 2)
                av(SKT - 1, stop=True)

                # Reciprocals of the denominator rows straight out of PSUM,
                # then raw attn copies (releases av banks); bp broadcast +
                # multiply deferred to early next pair via `pending`.
                # attn copies first (release av banks for the next pair),
                # then den->recip->f32r-rounding chain, all eager on DVE.
                # reciprocal_approx_fast is a bitwise custom-DVE op and CANNOT
                # read PSUM (garbage bits) -- den must bounce through SBUF.
                # On the final pair dens go first: no next pair to unblock,
                # and the drain's bp matmuls wait on the rec chain.
                halves = ((0, av_a), (1, av_b))
                dens = []

                def den_copies():
                    for half, avt in halves:
                        den = recp.tile([1, SC], F32, tag="den", name="den")
                        nc.vector.tensor_copy(den, avt[64:65, :])
                        dens.append(den)

                if last:
                    den_copies()
                for half, avt in halves:
                    attn_slice = attn_sb[j][bass.ds(64 * half, 64), sq]
                    nc.vector.tensor_copy(attn_slice, avt[0:64, :])
                if not last:
                    den_copies()
                for half in (0, 1):
                    rec32 = recp.tile([1, SC], F32, tag="rec32", name="rec32")
                    nc.vector.reciprocal_approx_fast(rec32, dens[half])
                    rec = recp.tile([1, SC], F32R, tag="rec")
                    nc.vector.tensor_copy(rec, rec32)
                    attn_slice = attn_sb[j][bass.ds(64 * half, 64), sq]
                    pending.append(make_normalize(attn_slice, rec, half))

            def make_oproj_filler(cs, st_local):
                """Returns a per-jj filler that emits o_proj work for sq-subtile
                st_local of chunk cs, one dm-chunk (4 mms + copy) at a time,
                spread over the 16-iteration attention j-loop."""
                st = cs * 4 + st_local
                ot = ostg.tile([128, D], F32, tag="ostg", name="ostg")
                state = {"mc": 0}

                def filler(jj):
                    # emit one dm-chunk at jj 5/8/11/14 (after the normalize
                    # pops at jj 3/4, so pair (cs-1, 3) attn is normalized
                    # before the first o_proj read)
                    if jj not in (6, 9, 12, 15) or state["mc"] >= D // SC:
                        return
                    mc = state["mc"]
                    state["mc"] += 1
                    op = mi_ps.tile([128, SC], F32, tag="mi", name="op")
                    for t in range(ET):
                        nc.tensor.matmul(
                            op,
                            attn_sb[t][:, bass.ts(st, 128)],
                            wo_sb[:, t, bass.ts(mc, SC)],
                            start=(t == 0),
                            stop=(t == ET - 1),
                        )
                    nc.vector.tensor_copy(ot[:, bass.ts(mc, SC)], op)
                    nc.sync.dma_start(
                        out=out[bass.ts(st, 128), bass.ts(mc, SC)],
                        in_=ot[:, bass.ts(mc, SC)],
                    )

                return filler

            for cs in range(NSC):
                for j in range(NPAIR):
                    filler = make_oproj_filler(cs - 1, j) if cs > 0 else None
                    attention(cs, j, pe_filler=filler,
                              last=(cs == NSC - 1 and j == NPAIR - 1))
            while pending:
                pending.pop(0)()
            for j in range(NPAIR):
                filler = make_oproj_filler(NSC - 1, j)
                for jj in range(SKT):
                    filler(jj)

    nc.compile()
    return nc


_PROGRAM = None


def _get_program():
    global _PROGRAM
    if _PROGRAM is None:
        _PROGRAM = build_program()
    return _PROGRAM


def _rope_tables():
    inv_freq = 1.0 / (ROPE_BASE ** (np.arange(0, HD, 2, dtype=np.float32) / HD))
    t = np.arange(S, dtype=np.float32)
    freqs = np.outer(t, inv_freq)  # [S, 32]
    emb = np.concatenate([freqs, freqs], axis=-1)  # [S, 64]
    return np.cos(emb).astype(np.float32), np.sin(emb).astype(np.float32)


# Feature permutation within each 64-wide head block: partition p holds
# feature PERM64[p]. Chosen so the RoPE pair (f, f+32) lands 16 partitions
# apart within one 32-partition quadrant (stream_shuffle constraint).
PERM64 = np.array(
    [p if p < 16 else p + 16 if p < 32 else p - 16 if p < 48 else p
     for p in range(64)]
)


def _host_constants():
    cos_t, sin_t = _rope_tables()  # [S, 64]
    idx = PERM64[np.arange(128) % HD]
    # rotate sign for the feature at partition p: rot(q)[f] = -q[f+32] for
    # f%64<32 (else +q[f-32]); with this layout that is p%32 < 16.
    sign = np.where(np.arange(128) % 32 < 16, -1.0, 1.0).astype(np.float32)
    cosd = np.ascontiguousarray(cos_t[:, idx].T)  # [128, S]
    sind = np.ascontiguousarray(sin_t[:, idx].T) * sign[:, None]

    ident = np.eye(128, dtype=np.float32)
    ones1 = np.ones((1, 128), np.float32)
    onesc = np.ones((128, 1), ml_dtypes.bfloat16)
    return cosd, sind, ident, ones1, onesc


def _core_inputs(x, Wq, Wk, Wv, Wo, consts, xt_by_batch, core):
    b, g = divmod(core, 4)
    cosd, sind, ident, ones1, onesc = consts

    wq_c = np.empty((D, E), np.float32)
    wo_c = np.empty((E, D), np.float32)
    for j in range(NPAIR):
        ha = 8 * g + j  # global head, kv-head 2g
        hb = 8 * g + j + 4  # global head, kv-head 2g+1
        # PERM64: q/k feature layout permuted per head (see _host_constants);
        # scores are invariant since q and k use the same permutation.
        wq_c[:, j * 128 : j * 128 + 64] = Wq[:, ha * HD + PERM64]
        wq_c[:, j * 128 + 64 : (j + 1) * 128] = Wq[:, hb * HD + PERM64]
        wo_c[j * 128 : j * 128 + 64, :] = Wo[ha * HD : (ha + 1) * HD, :]
        wo_c[j * 128 + 64 : (j + 1) * 128, :] = Wo[hb * HD : (hb + 1) * HD, :]
    kv0 = 2 * g * HD
    wk_c = np.concatenate(
        [Wk[:, kv0 + PERM64], Wk[:, kv0 + HD + PERM64]], axis=1
    )
    wv_c = np.ascontiguousarray(Wv[:, kv0 : kv0 + KVW])

    bf = ml_dtypes.bfloat16
    return {
        "xt": xt_by_batch[b],
        "wq": wq_c.astype(bf),
        "wk": wk_c.astype(bf),
        "wv": wv_c.astype(bf),
        "wo": wo_c.astype(bf),
        "cosd": cosd,
        "sind": sind,
        "ident": ident,
        "ones1": ones1,
        "onesc": onesc,
    }


def make_in_maps(x, Wq, Wk, Wv, Wo):
    consts = _host_constants()
    xt_by_batch = [
        np.ascontiguousarray(x[b].T).astype(ml_dtypes.bfloat16) for b in range(B)
    ]
    return [
        _core_inputs(x, Wq, Wk, Wv, Wo, consts, xt_by_batch, c)
        for c in range(N_CORES)
    ]


def kernel(x, Wq, Wk, Wv, Wo, _trace=False, _trace_kwargs=None):
    x = np.asarray(x, np.float32)
    Wq = np.asarray(Wq, np.float32)
    Wk = np.asarray(Wk, np.float32)
    Wv = np.asarray(Wv, np.float32)
    Wo = np.asarray(Wo, np.float32)

    nc = _get_program()
    in_maps = make_in_maps(x, Wq, Wk, Wv, Wo)
    res = bass_utils.run_bass_kernel_spmd(
        nc,
        in_maps,
        core_ids=list(range(N_CORES)),
        trace=_trace,
        **(_trace_kwargs or {}),
    )
    outs = [r["out"] for r in res.results]
    full = np.empty((B, S, D), np.float32)
    for b in range(B):
        full[b] = outs[4 * b] + outs[4 * b + 1] + outs[4 * b + 2] + outs[4 * b + 3]
    if _trace:
        return full, res
    return full



# revision 30
# speedup vs baseline: 1.3160x; 1.0025x over previous
"""GQA attention block (Wq/Wk/Wv -> RoPE -> softmax(QK^T)V -> Wo) on 8 Trainium2
NeuronCores.

Sharding (tensor-parallel per the head-sharding scheme):
  core c in 0..7: batch b = c // 4, head-group g = c % 4.
  Each core owns 8 q-heads (global 8g..8g+7) and 2 kv-heads (2g, 2g+1) of one
  batch element, computes its slice of q/k/v projections, RoPE, attention, and
  a partial o_proj (Wo rows for its heads). The all-reduce after o_proj is the
  host-side unshard: out[b] = sum of the 4 partial outputs of batch b.

On-device layout (per core), everything feature-on-partitions ("transposed"):
  xt    [D=2048, S=2048]   x^T for this batch
  QT    [E=512,  S]        q^T; partition-tile j holds head pair (j, j+4):
                           local head j (kv0) on partitions 0:64, head j+4
                           (kv1) on partitions 64:128. Wq columns are permuted
                           on the host to produce this layout directly.
  KT    [128, S]           k^T; kv0 on partitions 0:64, kv1 on 64:128.
  V     [S, 130] as 16 tiles [128, 130]: cols 0:64 v(kv0), col 64 ones,
                           cols 65:129 v(kv1), col 129 ones  (v_aug).
  scores^T per head: [sk, sq] so exp is ACT psum->sbuf and the attn@v
  contraction (over sk) uses v_aug as the stationary operand; row 64 of the
  attn@v output is the softmax denominator (ones column trick).

Matmuls run as float32r (full-rate fp32 on the PE; ~1.6e-4 rel err measured).
"""

import sys

if "/opt/trn_rl_repo" not in sys.path:
    sys.path.insert(0, "/opt/trn_rl_repo")

from contextlib import ExitStack

import numpy as np
import ml_dtypes

import concourse.bass as bass  # noqa: F401  (engine types via nc)
import concourse.tile as tile
from concourse import bacc, bass_utils, mybir

F32 = mybir.dt.float32
F32R = mybir.dt.float32r
BF16 = mybir.dt.bfloat16
AF = mybir.ActivationFunctionType

# Problem constants (hardcoded per harness contract)
B = 2
S = 2048  # sequence length
D = 2048  # d_model
N_HEADS = 32
N_KV = 8
HD = 64  # head dim
ROPE_BASE = 500000.0
N_CORES = 8

# Per-core derived
NQ = N_HEADS // 4  # 8 local q heads (4 head-groups)
E = NQ * HD  # 512 local q features
NPAIR = NQ // 2  # 4 head pairs / e-tiles
KVW = 2 * HD  # 128 local kv features
SC = 512  # s-chunk (projection + sq chunk)
NSC = S // SC  # 4
DT = D // 128  # 16 d-tiles
SKT = S // 128  # 16 sk tiles
ET = E // 128  # 4 e-tiles
SCALE = 1.0 / float(np.sqrt(HD))


def build_program():
    nc = bacc.Bacc(
        "TRN2", target_bir_lowering=False, debug=False, enable_asserts=False
    )

    xt = nc.dram_tensor("xt", [D, S], BF16, kind="ExternalInput").ap()
    wq = nc.dram_tensor("wq", [D, E], BF16, kind="ExternalInput").ap()
    wk = nc.dram_tensor("wk", [D, KVW], BF16, kind="ExternalInput").ap()
    wv = nc.dram_tensor("wv", [D, KVW], BF16, kind="ExternalInput").ap()
    wo = nc.dram_tensor("wo", [E, D], BF16, kind="ExternalInput").ap()
    cosd = nc.dram_tensor("cosd", [128, S], F32R, kind="ExternalInput").ap()
    sind = nc.dram_tensor("sind", [128, S], F32R, kind="ExternalInput").ap()
    ident = nc.dram_tensor("ident", [128, 128], F32, kind="ExternalInput").ap()
    ones1 = nc.dram_tensor("ones1", [1, 128], F32R, kind="ExternalInput").ap()
    onesc = nc.dram_tensor("onesc", [128, 1], BF16, kind="ExternalInput").ap()
    out = nc.dram_tensor("out", [S, D], F32, kind="ExternalOutput").ap()

    with tile.TileContext(nc) as tc, ExitStack() as ctx:
        persist = ctx.enter_context(tc.tile_pool(name="persist", bufs=1))

        # Persistent SBUF state
        qt_sb = [persist.tile([128, S], BF16, tag=f"qt{j}", name=f"qt{j}") for j in range(NPAIR)]
        kt_sb = persist.tile([128, S], BF16, tag="kt")
        v_sb = [persist.tile([128, 130], BF16, tag=f"v{j}", name=f"v{j}") for j in range(SKT)]
        onesc_sb = persist.tile([128, 1], BF16, tag="onesc")
        nc.sync.dma_start(out=onesc_sb, in_=onesc)
        ones1_sb = persist.tile([1, 128], F32R, tag="ones1")
        nc.sync.dma_start(out=ones1_sb, in_=ones1)

        # ---------------- Phase 1: projections + RoPE + V transpose -------------
        with (
            tc.tile_pool(name="xtp", bufs=2) as xtp,
            tc.tile_pool(name="wp", bufs=1) as wp,
            tc.tile_pool(name="ropec", bufs=1) as ropec,
            tc.tile_pool(name="p1st", bufs=3) as p1st,
            tc.tile_pool(name="qt_ps", bufs=2, space="PSUM") as qt_ps,
            tc.tile_pool(name="kv_ps", bufs=1, space="PSUM") as kv_ps,
            tc.tile_pool(name="tr_ps", bufs=2, space="PSUM") as tr_ps,
        ):

            wq_r = wq.rearrange("(t p) e -> p t e", p=128)
            wk_r = wk.rearrange("(t p) e -> p t e", p=128)
            wv_r = wv.rearrange("(t p) e -> p t e", p=128)
            wq_sb = wp.tile([128, DT, E], BF16, tag="wq")
            wk_sb = wp.tile([128, DT, KVW], BF16, tag="wk")
            wv_sb = wp.tile([128, DT, KVW], BF16, tag="wv")
            xt_r = xt.rearrange("(t p) s -> p t s", p=128)
            xt_c0 = xtp.tile([128, DT, SC], BF16, tag="xt", name="xt_c0")
            # wq via gpsimd SWDGE immediately (own queue): Q-proj of chunk 0
            # starts ~12us in and must not queue behind cos/sin on the ACT ring.
            # wv first (0.25MB, V-proj of chunk 0 needs it ~8us in), then
            # the 2MB wq (Q-proj starts ~25us in)
            nc.gpsimd.dma_start(out=wv_sb, in_=wv_r)
            nc.gpsimd.dma_start(out=wq_sb, in_=wq_r)
            ident_sb = ropec.tile([128, 128], F32, tag="ident")
            nc.scalar.dma_start(out=ident_sb, in_=ident)
            cos_sb = ropec.tile([128, S], F32R, tag="cos")
            nc.scalar.dma_start(out=cos_sb, in_=cosd)
            sin_sb = ropec.tile([128, S], F32R, tag="sin")
            nc.scalar.dma_start(out=sin_sb, in_=sind)
            # xt+wk interleaved per-tile: K-projection of chunk 0 unblocks
            # fastest.
            for t in range(DT):
                nc.sync.dma_start(out=xt_c0[:, t, :], in_=xt_r[:, t, 0:SC])
                nc.sync.dma_start(out=wk_sb[:, t, :], in_=wk_r[:, t, :])


            SHUF_MASK = [(i + 16) % 32 for i in range(32)]

            def rope(dst, src_ps, cs, raw_tag):
                """dst[:, cs*SC:+SC] = src_ps*cos + shuffle(src)*sin_signed.

                Features are laid out (host-side permutation) so the RoPE
                rotate pairing is a +-16 swap within each 32-partition
                quadrant; the rotate sign is folded into sind."""
                sl = bass.ts(cs, SC)
                raw = p1st.tile([128, SC], F32R, tag="raw", name="raw", bufs=3)
                nc.scalar.copy(raw, src_ps)
                rp = p1st.tile([128, SC], F32, tag="shuf", name="shuf", bufs=3)
                nc.vector.stream_shuffle(rp, raw, SHUF_MASK)
                tcos = p1st.tile([128, SC], F32, tag="tmp", name="tcos", bufs=4)
                nc.vector.tensor_mul(tcos, raw, cos_sb[:, sl])
                tsin = p1st.tile([128, SC], F32, tag="tmp", name="tsin", bufs=4)
                nc.vector.tensor_mul(tsin, rp, sin_sb[:, sl])
                nc.vector.tensor_add(dst[:, sl], tcos, tsin)

            for cs in range(NSC):
                if cs == 0:
                    xt_t = xt_c0
                else:
                    # single batched DMA per chunk: latency hidden behind
                    # chunk cs-1 compute, 1/16th the SP sequencer issue cost
                    xt_t = xtp.tile([128, DT, SC], BF16, tag="xt")
                    nc.sync.dma_start(
                        out=xt_t, in_=xt_r[:, :, bass.ts(cs, SC)]
                    )

                # KT projection + rope
                kp = kv_ps.tile([128, SC], F32, tag="kt")
                for t in range(DT):
                    nc.tensor.matmul(
                        kp,
                        wk_sb[:, t, :],
                        xt_t[:, t, :],
                        start=(t == 0),
                        stop=(t == DT - 1),
                    )
                rope(kt_sb, kp, cs, "kraw")

                # V^T projection, then transpose 128-subtiles into v_sb
                vp = kv_ps.tile([128, SC], F32, tag="vt")
                for t in range(DT):
                    nc.tensor.matmul(
                        vp,
                        wv_sb[:, t, :],
                        xt_t[:, t, :],
                        start=(t == 0),
                        stop=(t == DT - 1),
                    )
                vt_sb = p1st.tile([128, SC], F32, tag="vtsb", bufs=2)
                nc.vector.tensor_copy(vt_sb, vp)
                for ss in range(SC // 128):
                    sk = cs * (SC // 128) + ss
                    tp = tr_ps.tile([128, 128], F32, tag="tr")
                    nc.tensor.transpose(tp, vt_sb[:, bass.ts(ss, 128)], ident_sb)
                    nc.vector.tensor_copy(v_sb[sk][:, 0:64], tp[:, 0:64])
                    nc.vector.tensor_copy(v_sb[sk][:, 65:129], tp[:, 64:128])
                    nc.gpsimd.tensor_copy(v_sb[sk][:, 64:65], onesc_sb)
                    nc.gpsimd.tensor_copy(v_sb[sk][:, 129:130], onesc_sb)

                # QT projection + rope, per e-tile (head pair)
                for j in range(NPAIR):
                    qp = qt_ps.tile([128, SC], F32, tag="qt")
                    for t in range(DT):
                        nc.tensor.matmul(
                            qp,
                            wq_sb[:, t, bass.ts(j, 128)],
                            xt_t[:, t, :],
                            start=(t == 0),
                            stop=(t == DT - 1),
                        )
                    rope(qt_sb[j], qp, cs, "qraw")

        # ---------------- Phase 2 + 3: attention + o_proj ------------------------
        with (
            tc.tile_pool(name="wop", bufs=1) as wop,
            tc.tile_pool(name="attnp", bufs=1) as attnp,
            tc.tile_pool(name="expp", bufs=6) as expp,
            tc.tile_pool(name="recp", bufs=4) as recp,
            tc.tile_pool(name="ostg", bufs=3) as ostg,
            tc.tile_pool(name="sc_ps", bufs=2, space="PSUM") as sc_ps,
            tc.tile_pool(name="av_ps", bufs=1, space="PSUM") as av_ps,
            tc.tile_pool(name="mi_ps", bufs=2, space="PSUM") as mi_ps,
        ):
            wo_sb = wop.tile([128, ET, D], BF16, tag="wo")
            nc.gpsimd.dma_start(out=wo_sb, in_=wo.rearrange("(t p) d -> p t d", p=128))
            attn_sb = [attnp.tile([128, S], BF16, tag=f"at{j}", name=f"at{j}") for j in range(NPAIR)]

            pending = []

            def make_normalize(attn_slice, rec, half):
                # rec was computed eagerly at pair end (reciprocal straight
                # from the PSUM denominator row + f32r rounding copy), so the
                # bp matmul popped mid-next-pair does not stall the PE stream.
                def run():
                    bp = mi_ps.tile([128, SC], F32, tag="mi", name="bp")
                    nc.tensor.matmul(bp, ones1_sb, rec, start=True, stop=True)
                    nc.vector.tensor_mul(
                        attn_slice, attn_slice, bp[bass.ds(64 * half, 64), :]
                    )

                return run

            def attention(cs, j, pe_filler=None, last=False):
                """Head pair j (local heads j on kv0, j+4 on kv1), sq chunk cs."""
                sq = bass.ts(cs, SC)
                av_a = av_ps.tile([65, SC], F32, tag="ava")
                av_b = av_ps.tile([65, SC], F32, tag="avb")
                sc_t = [None, None]
                exp_t = [None] * SKT

                def scores(jj):
                    t = sc_ps.tile([128, 2 * SC], F32, tag="sc", name="sc")
                    sc_t[jj % 2] = t
                    nc.tensor.matmul(
                        t[:, 0:SC],
                        kt_sb[0:64, bass.ts(jj, 128)],
                        qt_sb[j][0:64, sq],
                        start=True,
                        stop=True,
                        tile_position=(0, 0),
                    )
                    nc.tensor.matmul(
                        t[:, SC : 2 * SC],
                        kt_sb[64:128, bass.ts(jj, 128)],
                        qt_sb[j][64:128, sq],
                        start=True,
                        stop=True,
                        tile_position=(64, 0),
                    )

                def av(t, start=False, stop=False):
                    nc.tensor.matmul(
                        av_a,
                        v_sb[t][:, 0:65],
                        exp_t[t][:, 0:SC],
                        start=start,
                        stop=stop,
                    )
                    nc.tensor.matmul(
                        av_b,
                        v_sb[t][:, 65:130],
                        exp_t[t][:, SC : 2 * SC],
                        start=start,
                        stop=stop,
                    )

                # AV runs two iterations behind its exp so the PE stream
                # rarely blocks on ACT latency.
                scores(0)
                for jj in range(SKT):
                    et = expp.tile([128, 2 * SC], BF16, tag="exp")
                    exp_t[jj] = et
                    nc.scalar.activation(et, sc_t[jj % 2], AF.Exp, scale=SCALE)
                    if jj + 1 < SKT:
                        scores(jj + 1)
                    if jj >= 2:
                        av(jj - 2, start=(jj == 2))
                    if jj in (4, 6) and pending:
                        pending.pop(0)()
                    if pe_filler is not None:
                        pe_filler(jj)
                av(SKT - 2)
                av(SKT - 1, stop=True)

                # Reciprocals of the denominator rows straight out of PSUM,
                # then raw attn copies (releases av banks); bp broadcast +
                # multiply deferred to early next pair via `pending`.
                # attn copies first (release av banks for the next pair),
                # then den->recip->f32r-rounding chain, all eager on DVE.
                # reciprocal_approx_fast is a bitwise custom-DVE op and CANNOT
                # read PSUM (garbage bits) -- den must bounce through SBUF.
                # On the final pair dens go first: no next pair to unblock,
                # and the drain's bp matmuls wait on the rec chain.
                halves = ((0, av_a), (1, av_b))
                dens = []

                def den_copies():
                    for half, avt in halves:
                        den = recp.tile([1, SC], F32, tag="den", name="den")
                        nc.vector.tensor_copy(den, avt[64:65, :])
                        dens.append(den)

                if last:
                    den_copies()
                for half, avt in halves:
                    attn_slice = attn_sb[j][bass.ds(64 * half, 64), sq]
                    nc.vector.tensor_copy(attn_slice, avt[0:64, :])
                if not last:
                    den_copies()
                for half in (0, 1):
                    rec32 = recp.tile([1, SC], F32, tag="rec32", name="rec32")
                    nc.vector.reciprocal_approx_fast(rec32, dens[half])
                    rec = recp.tile([1, SC], F32R, tag="rec")
                    nc.vector.tensor_copy(rec, rec32)
                    attn_slice = attn_sb[j][bass.ds(64 * half, 64), sq]
                    pending.append(make_normalize(attn_slice, rec, half))

            def make_oproj_filler(cs, st_local):
                """Returns a per-jj filler that emits o_proj work for sq-subtile
                st_local of chunk cs, one dm-chunk (4 mms + copy) at a time,
                spread over the 16-iteration attention j-loop."""
                st = cs * 4 + st_local
                ot = ostg.tile([128, D], F32, tag="ostg", name="ostg")
                state = {"mc": 0}

                def filler(jj):
                    # emit one dm-chunk at jj 5/8/11/14 (after the normalize
                    # pops at jj 3/4, so pair (cs-1, 3) attn is normalized
                    # before the first o_proj read)
                    if jj not in (6, 9, 12, 15) or state["mc"] >= D // SC:
                        return
                    mc = state["mc"]
                    state["mc"] += 1
                    op = mi_ps.tile([128, SC], F32, tag="mi", name="op")
                    for t in range(ET):
                        nc.tensor.matmul(
                            op,
                            attn_sb[t][:, bass.ts(st, 128)],
                            wo_sb[:, t, bass.ts(mc, SC)],
                            start=(t == 0),
                            stop=(t == ET - 1),
                        )
                    nc.vector.tensor_copy(ot[:, bass.ts(mc, SC)], op)
                    nc.sync.dma_start(
                        out=out[bass.ts(st, 128), bass.ts(mc, SC)],
                        in_=ot[:, bass.ts(mc, SC)],
                    )

                return filler

            for cs in range(NSC):
                for j in range(NPAIR):
                    filler = make_oproj_filler(cs - 1, j) if cs > 0 else None
                    attention(cs, j, pe_filler=filler,
                              last=(cs == NSC - 1 and j == NPAIR - 1))
            while pending:
                pending.pop(0)()
            for j in range(NPAIR):
                filler = make_oproj_filler(NSC - 1, j)
                for jj in range(SKT):
                    filler(jj)

    nc.compile()
    return nc


_PROGRAM = None


def _get_program():
    global _PROGRAM
    if _PROGRAM is None:
        _PROGRAM = build_program()
    return _PROGRAM


def _rope_tables():
    inv_freq = 1.0 / (ROPE_BASE ** (np.arange(0, HD, 2, dtype=np.float32) / HD))
    t = np.arange(S, dtype=np.float32)
    freqs = np.outer(t, inv_freq)  # [S, 32]
    emb = np.concatenate([freqs, freqs], axis=-1)  # [S, 64]
    return np.cos(emb).astype(np.float32), np.sin(emb).astype(np.float32)


# Feature permutation within each 64-wide head block: partition p holds
# feature PERM64[p]. Chosen so the RoPE pair (f, f+32) lands 16 partitions
# apart within one 32-partition quadrant (stream_shuffle constraint).
PERM64 = np.array(
    [p if p < 16 else p + 16 if p < 32 else p - 16 if p < 48 else p
     for p in range(64)]
)


def _host_constants():
    cos_t, sin_t = _rope_tables()  # [S, 64]
    idx = PERM64[np.arange(128) % HD]
    # rotate sign for the feature at partition p: rot(q)[f] = -q[f+32] for
    # f%64<32 (else +q[f-32]); with this layout that is p%32 < 16.
    sign = np.where(np.arange(128) % 32 < 16, -1.0, 1.0).astype(np.float32)
    cosd = np.ascontiguousarray(cos_t[:, idx].T)  # [128, S]
    sind = np.ascontiguousarray(sin_t[:, idx].T) * sign[:, None]

    ident = np.eye(128, dtype=np.float32)
    ones1 = np.ones((1, 128), np.float32)
    onesc = np.ones((128, 1), ml_dtypes.bfloat16)
    return cosd, sind, ident, ones1, onesc


def _core_inputs(x, Wq, Wk, Wv, Wo, consts, xt_by_batch, core):
    b, g = divmod(core, 4)
    cosd, sind, ident, ones1, onesc = consts

    wq_c = np.empty((D, E), np.float32)
    wo_c = np.empty((E, D), np.float32)
    for j in range(NPAIR):
        ha = 8 * g + j  # global head, kv-head 2g
        hb = 8 * g + j + 4  # global head, kv-head 2g+1
        # PERM64: q/k feature layout permuted per head (see _host_constants);
        # scores are invariant since q and k use the same permutation.
        wq_c[:, j * 128 : j * 128 + 64] = Wq[:, ha * HD + PERM64]
        wq_c[:, j * 128 + 64 : (j + 1) * 128] = Wq[:, hb * HD + PERM64]
        wo_c[j * 128 : j * 128 + 64, :] = Wo[ha * HD : (ha + 1) * HD, :]
        wo_c[j * 128 + 64 : (j + 1) * 128, :] = Wo[hb * HD : (hb + 1) * HD, :]
    kv0 = 2 * g * HD
    wk_c = np.concatenate(
        [Wk[:, kv0 + PERM64], Wk[:, kv0 + HD + PERM64]], axis=1
    )
    wv_c = np.ascontiguousarray(Wv[:, kv0 : kv0 + KVW])

    bf = ml_dtypes.bfloat16
    return {
        "xt": xt_by_batch[b],
        "wq": wq_c.astype(bf),
        "wk": wk_c.astype(bf),
        "wv": wv_c.astype(bf),
        "wo": wo_c.astype(bf),
        "cosd": cosd,
        "sind": sind,
        "ident": ident,
        "ones1": ones1,
        "onesc": onesc,
    }


def make_in_maps(x, Wq, Wk, Wv, Wo):
    consts = _host_constants()
    xt_by_batch = [
        np.ascontiguousarray(x[b].T).astype(ml_dtypes.bfloat16) for b in range(B)
    ]
    return [
        _core_inputs(x, Wq, Wk, Wv, Wo, consts, xt_by_batch, c)
        for c in range(N_CORES)
    ]


def kernel(x, Wq, Wk, Wv, Wo, _trace=False, _trace_kwargs=None):
    x = np.asarray(x, np.float32)
    Wq = np.asarray(Wq, np.float32)
    Wk = np.asarray(Wk, np.float32)
    Wv = np.asarray(Wv, np.float32)
    Wo = np.asarray(Wo, np.float32)

    nc = _get_program()
    in_maps = make_in_maps(x, Wq, Wk, Wv, Wo)
    res = bass_utils.run_bass_kernel_spmd(
        nc,
        in_maps,
        core_ids=list(range(N_CORES)),
        trace=_trace,
        **(_trace_kwargs or {}),
    )
    outs = [r["out"] for r in res.results]
    full = np.empty((B, S, D), np.float32)
    for b in range(B):
        full[b] = outs[4 * b] + outs[4 * b + 1] + outs[4 * b + 2] + outs[4 * b + 3]
    if _trace:
        return full, res
    return full

